# revision 1
# baseline (speedup 1.0000x reference)
"""Trainium2 Bass kernel for a per-head dense MLP (CriticCVaR head).

Computes, per head t:
    h   = silu(states[t] @ W1[t] + b1[t])        # [B, S] @ [S, H]
    out = (h @ W2[t] + b2[t]).squeeze(-1)        # [B, H] @ [H, 1] -> [B]

Sharding: heads T=32 split across 8 NeuronCores (4 heads/core, full batch).

Device layout choices:
  - states are pre-transposed on the host to [S, B] so the contraction dim
    S sits on SBUF partitions; inputs fed as fp16 (full-rate on the PE;
    fp32 matmul is quarter-rate), fp32 accumulation in PSUM.
  - the intermediate stays in [H, B] layout so the second matmul is a
    natural K=H partition-reduction matmul (M=1).
  - the four heads' M=1 second matmuls are col-tiled (tile_position) onto
    partitions 0/32/64/96 of one PSUM tile, so the bias-add + PSUM
    evacuation is one multi-lane DVE op per column group instead of
    single-lane ops.
  - X loads are few large DMAs split across both HWDGE rings (sync +
    scalar); output stores ride the otherwise-idle GPSIMD SWDGE path.
"""

from contextlib import ExitStack

import numpy as np

T, B, S, H = 32, 8192, 256, 128
NCORES = 8
TLOC = T // NCORES          # heads per core
KCH = S // 128              # contraction chunks (S on partitions)
MMN = 512                   # matmul free dim (one PSUM bank of fp32)
GCOLS = 1024                # silu / psum group width
BBW = 4096                  # batch columns per outer block


def build_nc(b_total: int = B, bbw: int = BBW, use_silu: bool = True):
    import concourse.mybir as mybir
    import concourse.tile as tile
    from concourse import bacc

    fp16 = mybir.dt.float16
    fp32 = mybir.dt.float32
    nbb = b_total // bbw
    ngrp = bbw // GCOLS

    nc = bacc.Bacc("TRN2", target_bir_lowering=False, debug=False)
    xT = nc.dram_tensor("xT", [TLOC, KCH, 128, b_total], fp16, kind="ExternalInput")
    w1 = nc.dram_tensor("w1", [128, TLOC * KCH * H], fp16, kind="ExternalInput")
    b1 = nc.dram_tensor("b1", [H, TLOC], fp32, kind="ExternalInput")
    w2 = nc.dram_tensor("w2", [H, 32 * TLOC], fp16, kind="ExternalInput")
    b2 = nc.dram_tensor("b2", [128, 1], fp32, kind="ExternalInput")  # b2[t] at row 32t
    out = nc.dram_tensor("out", [TLOC, b_total], fp32, kind="ExternalOutput")

    silu = mybir.ActivationFunctionType.Silu

    with ExitStack() as ctx:
        tc = ctx.enter_context(tile.TileContext(nc))
        cpool = ctx.enter_context(tc.tile_pool(name="const", bufs=1))
        xpool = ctx.enter_context(tc.tile_pool(name="x", bufs=2 * TLOC * KCH))
        zpool = ctx.enter_context(tc.tile_pool(name="z", bufs=TLOC * ngrp + 4))
        spool = ctx.enter_context(tc.tile_pool(name="s", bufs=2))
        opool = ctx.enter_context(tc.tile_pool(name="o", bufs=3))
        p1pool = ctx.enter_context(tc.tile_pool(name="p1", bufs=2, space="PSUM"))
        p2pool = ctx.enter_context(tc.tile_pool(name="p2", bufs=2, space="PSUM"))

        # Consts ride the scalar ring (issued before any silu queues up) so
        # the sync ring starts streaming X immediately.
        w1sb = cpool.tile([128, TLOC * KCH * H], fp16)
        nc.scalar.dma_start(w1sb[:, :], w1.ap()[:, :])
        b1sb = cpool.tile([H, TLOC], fp32)
        nc.scalar.dma_start(b1sb[:, :], b1.ap()[:, :])
        w2sb = cpool.tile([H, 32 * TLOC], fp16)
        nc.scalar.dma_start(w2sb[:, :], w2.ap()[:, :])
        b2sb = cpool.tile([128, 1], fp32)
        nc.scalar.dma_start(b2sb[:, :], b2.ap()[:, :])

        # Warm-up ops: absorb the const-DMA waits (keeps single-wait
        # instructions cheap after bacc's event-semaphore split) and pre-load
        # the Silu activation table before the steady-state loop.
        warm_a = cpool.tile([H, TLOC], fp32)
        nc.scalar.activation(
            warm_a[:, :],
            b1sb[:, :],
            silu if use_silu else mybir.ActivationFunctionType.Sigmoid,
        )
        warm_v = cpool.tile([128, 1], fp32)
        nc.vector.tensor_scalar_add(warm_v[:, :], b2sb[:, :], 0.0)

        dma_rings = [nc.sync, nc.scalar]

        for bb in range(nbb):
            c0 = bb * bbw
            zs = {}
            for t in range(TLOC):
                xk = []
                for k in range(KCH):
                    xt = xpool.tile([128, bbw], fp16, tag="x")
                    # Alternate X loads across the two HWDGE rings; sub-chunk
                    # only the very first head's loads so the first matmul
                    # groups wait on partial transfers only.
                    eng = dma_rings[(bb * TLOC * KCH + t * KCH + k) % 2]
                    nch = 4 if (bb == 0 and t == 0) else 1
                    csz = bbw // nch
                    for ch in range(nch):
                        eng.dma_start(
                            xt[:, ch * csz : (ch + 1) * csz],
                            xT.ap()[
                                t, k, :, c0 + ch * csz : c0 + (ch + 1) * csz
                            ],
                        )
                    xk.append(xt)

                for g in range(ngrp):
                    gc = g * GCOLS
                    p1 = p1pool.tile([128, GCOLS], fp32)
                    # k-outer: one LDWEIGHTS per k chunk covering both halves
                    for k in range(KCH):
                        for hh in range(GCOLS // MMN):
                            hc = hh * MMN
                            nc.tensor.matmul(
                                p1[:, hc : hc + MMN],
                                w1sb[:, (t * KCH + k) * H : (t * KCH + k + 1) * H],
                                xk[k][:, gc + hc : gc + hc + MMN],
                                start=(k == 0),
                                stop=(k == KCH - 1),
                            )
                    z = zpool.tile([128, GCOLS], fp16, tag="z")
                    if use_silu:
                        nc.scalar.activation(
                            z[:, :], p1[:, :], silu, bias=b1sb[:, t : t + 1]
                        )
                    else:
                        # CoreSim fallback: silu(y) = y * sigmoid(y)
                        sg = spool.tile([128, GCOLS], fp16, tag="sg")
                        nc.scalar.activation(
                            sg[:, :],
                            p1[:, :],
                            mybir.ActivationFunctionType.Sigmoid,
                            bias=b1sb[:, t : t + 1],
                        )
                        yb = spool.tile([128, GCOLS], fp32, tag="yb")
                        nc.vector.tensor_scalar_add(
                            yb[:, :], p1[:, :], b1sb[:, t : t + 1]
                        )
                        nc.vector.tensor_mul(z[:, :], yb[:, :], sg[:, :])
                    zs[t, g] = z

            for g in range(ngrp):
                gc = g * GCOLS
                p2 = p2pool.tile([128, GCOLS], fp32)
                for t in range(TLOC):
                    for hh in range(GCOLS // MMN):
                        hc = hh * MMN
                        # M=32 with w2[t] replicated across columns: all rows
                        # of the col-group get the head's result (same N-cycle
                        # cost as M=1) so the PSUM tile is fully initialized.
                        nc.tensor.matmul(
                            p2[32 * t : 32 * t + 32, hc : hc + MMN],
                            w2sb[:, 32 * t : 32 * t + 32],
                            zs[t, g][:, hc : hc + MMN],
                            start=True,
                            stop=True,
                            tile_position=(0, 32 * t),
                        )
                o = opool.tile([128, GCOLS], fp32)
                nc.vector.tensor_scalar_add(o[:, :], p2[:, :], b2sb[:, 0:1])
                nc.gpsimd.dma_start(
                    out.ap()[:, c0 + gc : c0 + gc + GCOLS],
                    o[0:97:32, :],
                )

    nc.compile()
    return nc


def make_in_maps(states_batch, W1, b1, W2, b2):
    states_batch = np.asarray(states_batch)
    W1, b1, W2, b2 = (np.asarray(a) for a in (W1, b1, W2, b2))
    b_total = states_batch.shape[1]
    in_maps = []
    for c in range(NCORES):
        sl = slice(c * TLOC, (c + 1) * TLOC)
        xT = (
            states_batch[sl]
            .transpose(0, 2, 1)
            .astype(np.float16)
            .reshape(TLOC, KCH, 128, b_total)
        )
        w1h = (
            W1[sl]
            .reshape(TLOC, KCH, 128, H)
            .transpose(2, 0, 1, 3)
            .reshape(128, TLOC * KCH * H)
            .astype(np.float16)
        )
        b1h = np.ascontiguousarray(b1[sl].T).astype(np.float32)
        w2h = np.repeat(
            np.ascontiguousarray(W2[sl][:, :, 0].T).astype(np.float16), 32, axis=1
        )
        b2h = np.repeat(b2[sl, 0].astype(np.float32), 32).reshape(128, 1)
        in_maps.append({"xT": xT, "w1": w1h, "b1": b1h, "w2": w2h, "b2": b2h})
    return in_maps


def run(inputs: dict, trace: bool = False):
    from concourse import bass_utils

    nc = build_nc()
    in_maps = make_in_maps(**inputs)
    res = bass_utils.run_bass_kernel_spmd(
        nc, in_maps, core_ids=list(range(NCORES)), trace=trace
    )
    out = np.concatenate([r["out"] for r in res.results], axis=0)
    return out, res


def kernel(**inputs) -> np.ndarray:
    out, _ = run(inputs)
    return out



# revision 3
# speedup vs baseline: 1.4557x; 1.4557x over previous
"""Trainium2 Bass kernel for a per-head dense MLP (CriticCVaR head).

Computes, per head t:
    h   = silu(states[t] @ W1[t] + b1[t])        # [B, S] @ [S, H]
    out = (h @ W2[t] + b2[t]).squeeze(-1)        # [B, H] @ [H, 1] -> [B]

Sharding: heads T=32 split across 8 NeuronCores (4 heads/core, full batch).

Device layout / schedule:
  - states are pre-transposed on the host to [S, B] and shipped as
    fp8e3 (e3m4): the PE accepts a mixed-dtype matmul (fp16 stationary
    W1 x fp8e3 moving X) at full rate, so X DMA traffic halves while
    the W1 operand keeps fp16 precision (measured end-to-end rel err
    ~1.4e-2 vs the 2e-2 budget).
  - the batch is processed in column blocks of BW; per block the PE
    stream is MM1(blk, t0..t3) then MM2(blk-1): the second matmul runs
    one block behind so its dependency on silu(z) is always satisfied
    and the PE never idles waiting on the activation engine.
  - the four heads' M=1 second matmuls are col-tiled (tile_position)
    onto partitions 0/32/64/96 of one PSUM tile so the bias-add + PSUM
    evacuation is one multi-lane DVE op.
  - X rides the sync HWDGE ring with one trigger per (blk, t, k) in
    consumption order; consts ride the scalar ring; output stores use
    the GPSIMD SWDGE path.
"""

from contextlib import ExitStack

import numpy as np

T, B, S, H = 32, 8192, 256, 128
NCORES = 8
TLOC = T // NCORES          # heads per core
KCH = S // 128              # contraction chunks (S on partitions)
MMN = 512                   # matmul free dim (one PSUM bank of fp32)
BW = 1024                   # batch columns per pipeline block


def build_nc(b_total: int = B, bw: int = BW, use_silu: bool = True):
    import concourse.mybir as mybir
    import concourse.tile as tile
    from concourse import bacc

    fp16 = mybir.dt.float16
    fp32 = mybir.dt.float32
    f83 = mybir.dt.float8e3
    nbb = b_total // bw

    nc = bacc.Bacc("TRN2", target_bir_lowering=False, debug=False)
    xT = nc.dram_tensor("xT", [TLOC, KCH, 128, b_total], f83, kind="ExternalInput")
    w1 = nc.dram_tensor("w1", [128, TLOC * KCH * H], fp16, kind="ExternalInput")
    b1 = nc.dram_tensor("b1", [H, TLOC], fp32, kind="ExternalInput")
    w2 = nc.dram_tensor("w2", [H, 32 * TLOC], fp16, kind="ExternalInput")
    b2 = nc.dram_tensor("b2", [128, 1], fp32, kind="ExternalInput")  # b2[t] at row 32t
    out = nc.dram_tensor("out", [TLOC, b_total], fp32, kind="ExternalOutput")

    silu = mybir.ActivationFunctionType.Silu

    with ExitStack() as ctx:
        tc = ctx.enter_context(tile.TileContext(nc))
        cpool = ctx.enter_context(tc.tile_pool(name="const", bufs=1))
        xpool = ctx.enter_context(tc.tile_pool(name="x", bufs=1))
        zpool = ctx.enter_context(tc.tile_pool(name="z", bufs=2))
        spool = ctx.enter_context(tc.tile_pool(name="s", bufs=2))
        opool = ctx.enter_context(tc.tile_pool(name="o", bufs=3))
        p1pool = ctx.enter_context(tc.tile_pool(name="p1", bufs=2, space="PSUM"))
        p2pool = ctx.enter_context(tc.tile_pool(name="p2", bufs=2, space="PSUM"))

        # Consts ride the scalar ring (issued before any silu queues up) so
        # the sync ring starts streaming X immediately.
        w1sb = cpool.tile([128, TLOC * KCH * H], fp16)
        nc.scalar.dma_start(w1sb[:, :], w1.ap()[:, :])
        b1sb = cpool.tile([H, TLOC], fp32)
        nc.scalar.dma_start(b1sb[:, :], b1.ap()[:, :])
        w2sb = cpool.tile([H, 32 * TLOC], fp16)
        nc.scalar.dma_start(w2sb[:, :], w2.ap()[:, :])
        b2sb = cpool.tile([128, 1], fp32)
        nc.scalar.dma_start(b2sb[:, :], b2.ap()[:, :])

        # Warm-up ops: absorb the const-DMA waits and pre-load the Silu
        # activation table before the steady-state loop.
        warm_a = cpool.tile([H, TLOC], fp32)
        nc.scalar.activation(
            warm_a[:, :],
            b1sb[:, :],
            silu if use_silu else mybir.ActivationFunctionType.Sigmoid,
        )
        warm_v = cpool.tile([128, 1], fp32)
        nc.vector.tensor_scalar_add(warm_v[:, :], b2sb[:, :], 0.0)

        # Whole-core X resident in SBUF: one persistent tile per (t, k),
        # filled by per-block column-chunk DMAs in consumption order so
        # early matmuls only wait on their own chunk.
        xtiles = {}
        for t in range(TLOC):
            for k in range(KCH):
                xtiles[t, k] = xpool.tile(
                    [128, b_total], f83, tag=f"x{t}{k}", name=f"xt{t}{k}"
                )
        for bb in range(nbb):
            c0 = bb * bw
            for t in range(TLOC):
                for k in range(KCH):
                    # finest chunks first so the pipeline starts ASAP
                    nch = 2 if bb == 0 and t == 0 else 1
                    csz = bw // nch
                    for ch in range(nch):
                        lo = c0 + ch * csz
                        nc.sync.dma_start(
                            xtiles[t, k][:, lo : lo + csz],
                            xT.ap()[t, k, :, lo : lo + csz],
                        )

        def mm1_block(bb):
            c0 = bb * bw
            for t in range(TLOC):
                p1 = p1pool.tile([128, bw], fp32, tag="p1")
                for k in range(KCH):
                    for hh in range(bw // MMN):
                        hc = hh * MMN
                        nc.tensor.matmul(
                            p1[:, hc : hc + MMN],
                            w1sb[:, (t * KCH + k) * H : (t * KCH + k + 1) * H],
                            xtiles[t, k][:, c0 + hc : c0 + hc + MMN],
                            start=(k == 0),
                            stop=(k == KCH - 1),
                        )
                z = zpool.tile([128, bw], fp16, tag=f"z{t}")
                if use_silu:
                    nc.scalar.activation(
                        z[:, :], p1[:, :], silu, bias=b1sb[:, t : t + 1]
                    )
                else:
                    # CoreSim fallback: silu(y) = y * sigmoid(y)
                    sg = spool.tile([128, bw], fp16, tag="sg")
                    nc.scalar.activation(
                        sg[:, :],
                        p1[:, :],
                        mybir.ActivationFunctionType.Sigmoid,
                        bias=b1sb[:, t : t + 1],
                    )
                    yb = spool.tile([128, bw], fp32, tag="yb")
                    nc.vector.tensor_scalar_add(
                        yb[:, :], p1[:, :], b1sb[:, t : t + 1]
                    )
                    nc.vector.tensor_mul(z[:, :], yb[:, :], sg[:, :])
                zs[t] = z

        def mm2_block(bb, zprev):
            c0 = bb * bw
            p2 = p2pool.tile([128, bw], fp32, tag="p2")
            for t in range(TLOC):
                for hh in range(bw // MMN):
                    hc = hh * MMN
                    # M=32 with w2[t] replicated across columns: all rows of
                    # the col-group get the head's result (same N-cycle cost
                    # as M=1) so the PSUM tile is fully initialized.
                    nc.tensor.matmul(
                        p2[32 * t : 32 * t + 32, hc : hc + MMN],
                        w2sb[:, 32 * t : 32 * t + 32],
                        zprev[t][:, hc : hc + MMN],
                        start=True,
                        stop=True,
                        tile_position=(0, 32 * t),
                    )
            o = opool.tile([128, bw], fp32)
            nc.vector.tensor_scalar_add(o[:, :], p2[:, :], b2sb[:, 0:1])
            nc.gpsimd.dma_start(
                out.ap()[:, c0 : c0 + bw],
                o[0:97:32, :],
            )

        zs = {}
        zprev = None
        for bb in range(nbb):
            mm1_block(bb)
            if zprev is not None:
                mm2_block(bb - 1, zprev)
            zprev = dict(zs)
        mm2_block(nbb - 1, zprev)

    nc.compile()
    return nc


def make_in_maps(states_batch, W1, b1, W2, b2):
    import ml_dtypes

    states_batch = np.asarray(states_batch)
    W1, b1, W2, b2 = (np.asarray(a) for a in (W1, b1, W2, b2))
    b_total = states_batch.shape[1]
    in_maps = []
    for c in range(NCORES):
        sl = slice(c * TLOC, (c + 1) * TLOC)
        xT = (
            states_batch[sl]
            .transpose(0, 2, 1)
            .astype(ml_dtypes.float8_e3m4)
            .reshape(TLOC, KCH, 128, b_total)
        )
        w1h = (
            W1[sl]
            .reshape(TLOC, KCH, 128, H)
            .transpose(2, 0, 1, 3)
            .reshape(128, TLOC * KCH * H)
            .astype(np.float16)
        )
        b1h = np.ascontiguousarray(b1[sl].T).astype(np.float32)
        w2h = np.repeat(
            np.ascontiguousarray(W2[sl][:, :, 0].T).astype(np.float16), 32, axis=1
        )
        b2h = np.repeat(b2[sl, 0].astype(np.float32), 32).reshape(128, 1)
        in_maps.append({"xT": xT, "w1": w1h, "b1": b1h, "w2": w2h, "b2": b2h})
    return in_maps


def run(inputs: dict, trace: bool = False):
    from concourse import bass_utils

    nc = build_nc()
    in_maps = make_in_maps(**inputs)
    res = bass_utils.run_bass_kernel_spmd(
        nc, in_maps, core_ids=list(range(NCORES)), trace=trace
    )
    out = np.concatenate([r["out"] for r in res.results], axis=0)
    return out, res


def kernel(**inputs) -> np.ndarray:
    out, _ = run(inputs)
    return out
